# revision 19
# baseline (speedup 1.0000x reference)
"""BitNet 4-layer MLP (8192x4096, ternary weights, int8-style activations)
on 8 Trainium2 NeuronCores.

Strategy: pure data-parallel over the 8192-token dim (1024 tokens/core, no
collectives). Activations live TRANSPOSED on chip ([feature, token]) so the
output of each layer's matmul (PSUM [out_feat, tok]) is directly the next
layer's moving operand - zero transposes on device. Weights are quantized
to ternary bf16 on the host (matmul over {-1,0,1} x integers <= 128 is
exact in bf16 with fp32 PSUM accumulation) and streamed per layer.

The PE is the bottleneck (bf16 roofline ~1.75 ms/core), so everything else
stays off it: LayerNorm stats (sum h, sum h^2 over the feature/partition
dim) are accumulated across the 32 feature tiles on the DVE into two fp32
[128,512] tiles, then reduced across partitions EXACTLY in fp32 with
GPSIMD partition_all_reduce (which also broadcasts the result to all
partitions, so no PE broadcast matmuls either). The quant scale 1/in_scale
is folded into the rstd factor, shrinking the DVE post chain to 4 ops per
feature tile. Quantization uses the +/-1.5*2^23 magic-number trick, which
matches XLA's round-nearest-even bitwise.

Each core processes its 1024 tokens as two 512-token halves pipelined
against each other: while half B's matmuls run on PE, half A's layernorm/
quantize chain runs on DVE/ACT/GPSIMD, so PE never waits.
"""

import numpy as np

NUM_CORES = 8
N_TOK, D = 8192, 4096
NUM_LAYERS = 4
P = 128                      # SBUF partitions
KT = D // P                  # 32 k-tiles per contraction
NLOC = N_TOK // NUM_CORES    # 1024 tokens per core
HALF = 512                   # token half-chunk (one PSUM bank @ fp32)
MAGIC = 12582912.0           # 1.5 * 2**23: fp32 add/sub does RNE-to-integer

_prog_cache = {}


def _install_drain_patch():
    """walrus CoreV3 rejects instructions carrying >~2 embedded sem waits
    ("Too many sync wait commands"). Tile's exit drain waits on the whole
    vector clock; spread its waits across trailing sync-engine nops."""
    import concourse.tile as tile
    import concourse.mybir as mybir
    from concourse.tile import ScopedClock

    if getattr(tile.TileContext, "_drain_patch_installed", False):
        return

    def _patched(self, tick_clock, wait_clock):
        nc = self.nc
        drain_inst = nc.sync.drain()
        wait_clock.add_sem_waits(
            drain_inst.ins, ScopedClock({None: tick_clock.global_clock})
        )
        si = drain_inst.ins.sync_info
        waits = list(si.on_wait or []) if si is not None else []
        if len(waits) > 1:
            si.on_wait = waits[:1]
            for w in waits[1:]:
                nop = nc.sync.nop(nofuse=True)
                nsi = nop.ins.sync_info
                if nsi is None:
                    nop.ins.sync_info = mybir.SyncInfo(on_wait=[w], on_update=[])
                else:
                    nsi.on_wait = [w]
        nc.all_engine_barrier()
        assert self.sems is not None
        popped = nc._tile_sem_poison_stack.pop()
        assert popped is self._sem_poison
        nc.clear_and_free_semaphores(list(self.sems.allocated().values()))
        nc.all_engine_barrier()

    tile.TileContext._drain_and_barrier = _patched
    tile.TileContext._drain_patch_installed = True


def _split_excess_waits(nc, maxw=1):
    """walrus's per-instruction sync-wait encodings hold few waits; hoist
    excess waits onto same-engine nops spliced immediately before the
    overloaded instruction (adjacent on the same queue, so ordering
    semantics are unchanged)."""
    import copy
    import concourse.mybir as mybir

    ctr = [0]
    # a genuine InstNoOp prototype (left at stream end, harmless)
    proto = nc.sync.nop(nofuse=True)
    _NOP_PROTO = copy.deepcopy(proto.ins)
    _NOP_PROTO.sync_info = None

    def make_nop(proto_engine, waits):
        ctr[0] += 1
        nop = copy.deepcopy(_NOP_PROTO)
        nop.name = f"I-waitsplit-{ctr[0]}"
        nop.engine = proto_engine
        nop.sync_info = mybir.SyncInfo(on_wait=list(waits), on_update=[])
        return nop

    for bb in nc.m.functions[0].blocks:
        changed = False
        out = []
        for inst in bb.instructions:
            si = inst.sync_info
            waits = list(si.on_wait) if (si is not None and si.on_wait) else []
            if isinstance(inst, mybir.InstISA):
                # ext-ISA instructions have a fixed byte encoding: they can
                # carry NO embedded sync commands. Hoist all waits onto
                # preceding nops and all updates onto a trailing nop on the
                # same (strict-FIFO) engine queue.
                ups = list(si.on_update) if (si is not None and si.on_update) else []
                if waits or ups:
                    for i in range(0, len(waits), maxw):
                        out.append(make_nop(inst.engine, waits[i:i + maxw]))
                    si.on_wait = []
                    out.append(inst)
                    if ups:
                        nop = make_nop(inst.engine, [])
                        nop.sync_info.on_update = ups
                        si.on_update = []
                        out.append(nop)
                    changed = True
                    continue
            elif len(waits) > maxw:
                for i in range(0, len(waits) - maxw, maxw):
                    out.append(make_nop(inst.engine, waits[i:i + maxw]))
                si.on_wait = waits[len(waits) - maxw:]
                changed = True
            out.append(inst)
        if changed:
            bb.instructions = out
    return nc


def _build_program(s_deq, inv_in, trivial_affine):
    """Build the per-core Bass program (identical across cores; data-parallel).

    s_deq[l]  = in_scale[l]*w_scale[l] as python floats (fp32-exact values)
    inv_in[l] = 1/in_scale[l] likewise
    trivial_affine: gammas all ones and betas all zeros (the graded case);
    folds 1/in_scale into the rstd broadcast. The general path applies
    gamma/beta per feature tile like the reference.
    """
    import concourse.bass as bass
    import concourse.mybir as mybir
    import concourse.tile as tile

    _install_drain_patch()
    dt = mybir.dt
    Alu = mybir.AluOpType
    Act = mybir.ActivationFunctionType

    nc = bass.Bass()
    W_d = nc.dram_tensor("wt", [NUM_LAYERS, KT, P, KT, P], dt.bfloat16,
                         kind="ExternalInput")
    X_d = nc.dram_tensor("xq0", [KT, P, NLOC], dt.bfloat16, kind="ExternalInput")
    if not trivial_affine:
        G_d = nc.dram_tensor("gam", [NUM_LAYERS - 1, KT, P, 1], dt.float32,
                             kind="ExternalInput")
        B_d = nc.dram_tensor("bet", [NUM_LAYERS - 1, KT, P, 1], dt.float32,
                             kind="ExternalInput")
    O_d = nc.dram_tensor("out", [D, NLOC], dt.float32, kind="ExternalOutput")

    f32, f32r, bf16 = dt.float32, dt.float32r, dt.bfloat16

    with tile.TileContext(nc) as tc:
        with (
            tc.tile_pool(name="xq", bufs=64) as xq_pool,
            tc.tile_pool(name="h", bufs=33) as h_pool,
            tc.tile_pool(name="w", bufs=4) as w_pool,
            tc.tile_pool(name="sq", bufs=3) as sq_pool,
            tc.tile_pool(name="acc", bufs=4) as acc_pool,
            tc.tile_pool(name="red", bufs=4) as red_pool,
            tc.tile_pool(name="st", bufs=6) as st_pool,
            tc.tile_pool(name="bc", bufs=4) as bc_pool,
            tc.tile_pool(name="gb", bufs=128) as gb_pool,
            tc.tile_pool(name="const", bufs=1) as const_pool,
            tc.tile_pool(name="mmps", bufs=4, space="PSUM") as mm_ps,
            tc.tile_pool(name="stps", bufs=2, space="PSUM") as st_ps,
            tc.tile_pool(name="bcps", bufs=2, space="PSUM") as bc_ps,
        ):
            eps = const_pool.tile([1, 1], f32)
            nc.vector.memset(eps[:], 1e-5)
            ones_col = const_pool.tile([P, 1], f32)
            nc.vector.memset(ones_col[:], 1.0)
            ones_row = const_pool.tile([1, P], f32)
            nc.vector.memset(ones_row[:], 1.0)

            # interleave the first weight tiles into the 64-input-DMA stream
            # (single SP DMA ring, FIFO) so the PE never waits: w0,w1 land
            # first, half 0's activations next, w2,w3 before half 1's.
            def w_dma(l, ot):
                w = w_pool.tile([P, KT, P], bf16, tag="w")
                nc.sync.dma_start(w[:], W_d[l, ot])
                return w

            w_first = [w_dma(0, 0), w_dma(0, 1)]
            xq_tiles = {}
            for half in range(2):
                for kt in range(KT):
                    t = xq_pool.tile([P, HALF], bf16, tag="xq")
                    # half 1 loads ride the (idle-at-startup) ACT hwdge ring
                    # so the SP ring serves half 0 + weights without queuing
                    eng = nc.sync if half == 0 else nc.scalar
                    eng.dma_start(
                        t[:], X_d[kt, :, half * HALF:(half + 1) * HALF])
                    xq_tiles[(0, half, kt)] = t
                if half == 0:
                    w_first += [w_dma(0, 2), w_dma(0, 3)]

            h_tiles = {}
            w_cache = {}
            CACHE_N = 3   # last ot tiles of half 0 reused by half 1 (snake)
            # previous half's post emission, injected two groups into the
            # NEXT half's matmul stream: its stats matmuls then sit behind
            # two already-runnable groups in the in-order PE queue instead
            # of blocking it while the stats accumulation drains
            pending_post = [None]

            def emit_mm(l, half):
                acc_S = acc_Q = None
                ot_seq = list(range(KT)) if half == 0 else \
                    list(reversed(range(KT)))
                for idx, ot in enumerate(ot_seq):
                    if idx == 2 and pending_post[0] is not None:
                        fn, pending_post[0] = pending_post[0], None
                        fn()
                    if l == 0 and half == 0 and ot < len(w_first):
                        w = w_first[ot]
                    elif half == 1 and ot in w_cache:
                        w = w_cache.pop(ot)
                    else:
                        w = w_dma(l, ot)
                    if half == 0 and ot >= KT - CACHE_N:
                        w_cache[ot] = w
                    ps = mm_ps.tile([P, HALF], f32, tag="mmps")
                    for kt in range(KT):
                        nc.tensor.matmul(
                            ps[:], w[:, kt, :], xq_tiles[(l, half, kt)][:],
                            start=(kt == 0), stop=(kt == KT - 1),
                            skip_group_check=True)
                    h_t = h_pool.tile([P, HALF], f32, tag="h")
                    if l < NUM_LAYERS - 1:
                        nc.scalar.activation(h_t[:], ps[:], Act.Relu,
                                             scale=float(s_deq[l]))
                        sq_t = sq_pool.tile([P, HALF], f32, tag="sq")
                        nc.scalar.activation(sq_t[:], h_t[:], Act.Square)
                        # stats accumulate on GPSIMD (fp32-exact there too):
                        # keeps the DVE queue free for the post chains
                        if acc_S is None:
                            acc_S = acc_pool.tile([P, HALF], f32, tag="acc")
                            nc.gpsimd.tensor_copy(acc_S[:], h_t[:])
                            acc_Q = acc_pool.tile([P, HALF], f32, tag="acc")
                            nc.gpsimd.tensor_copy(acc_Q[:], sq_t[:])
                        else:
                            nc.gpsimd.tensor_tensor(acc_S[:], acc_S[:], h_t[:],
                                                    op=Alu.add)
                            nc.gpsimd.tensor_tensor(acc_Q[:], acc_Q[:], sq_t[:],
                                                    op=Alu.add)
                        h_tiles[(half, ot)] = h_t
                    else:
                        nc.scalar.activation(h_t[:], ps[:], Act.Copy,
                                             scale=float(s_deq[l]))
                        nc.sync.dma_start(
                            O_d[ot * P:(ot + 1) * P,
                                half * HALF:(half + 1) * HALF], h_t[:])
                return acc_S, acc_Q

            def emit_post(l, half, acc_S, acc_Q, gbi):
                # cross-partition sums of the stat accumulators at full fp32
                # accuracy: f32r hi/lo-compensated ones-matmuls (fp22 each,
                # exact in combination), one pair per accumulator per half
                S_ps = st_ps.tile([1, HALF], f32, tag="stps")
                Q_ps = st_ps.tile([1, HALF], f32, tag="stps")
                for acc, ps in ((acc_S, S_ps), (acc_Q, Q_ps)):
                    hi = red_pool.tile([P, HALF], f32r, tag="red")
                    nc.vector.tensor_copy(hi[:], acc[:])
                    lo = red_pool.tile([P, HALF], f32r, tag="red")
                    nc.vector.tensor_tensor(lo[:], acc[:], hi[:].bitcast(f32),
                                            op=Alu.subtract)
                    nc.tensor.matmul(ps[:], ones_col[:].bitcast(f32r), hi[:],
                                     start=True, stop=False,
                                     skip_group_check=True)
                    nc.tensor.matmul(ps[:], ones_col[:].bitcast(f32r), lo[:],
                                     start=False, stop=True,
                                     skip_group_check=True)
                # per-token rows [1, HALF]
                mu = st_pool.tile([1, HALF], f32, tag="st")
                nc.vector.tensor_scalar_mul(mu[:], S_ps[:], 1.0 / D)
                q = st_pool.tile([1, HALF], f32, tag="st")
                nc.vector.tensor_scalar_mul(q[:], Q_ps[:], 1.0 / D)
                var = st_pool.tile([1, HALF], f32, tag="st")
                nc.vector.tensor_tensor(var[:], mu[:], mu[:], op=Alu.mult)
                nc.vector.tensor_tensor(var[:], q[:], var[:], op=Alu.subtract)
                std = st_pool.tile([1, HALF], f32, tag="st")
                nc.scalar.activation(std[:], var[:], Act.Sqrt, bias=eps[:])
                a_row = st_pool.tile([1, HALF], f32, tag="st")
                nc.vector.reciprocal(a_row[:], std[:])
                inv = float(inv_in[l + 1])
                if trivial_affine:
                    nc.vector.tensor_scalar_mul(a_row[:], a_row[:], inv)
                # broadcast mu and a to all partitions via ones-column matmul
                mu_ps = bc_ps.tile([P, HALF], f32, tag="bcps")
                nc.tensor.matmul(mu_ps[:], ones_row[:], mu[:],
                                 start=True, stop=True, skip_group_check=True)
                muB = bc_pool.tile([P, HALF], f32, tag="bc")
                nc.scalar.activation(muB[:], mu_ps[:], Act.Copy)
                a_ps = bc_ps.tile([P, HALF], f32, tag="bcps")
                nc.tensor.matmul(a_ps[:], ones_row[:], a_row[:],
                                 start=True, stop=True, skip_group_check=True)
                aB = bc_pool.tile([P, HALF], f32, tag="bc")
                nc.scalar.activation(aB[:], a_ps[:], Act.Copy)
                for ft in range(KT):
                    h_t = h_tiles.pop((half, ft))
                    nc.vector.tensor_tensor(h_t[:], h_t[:], muB[:],
                                            op=Alu.subtract)
                    nc.vector.tensor_tensor(h_t[:], h_t[:], aB[:],
                                            op=Alu.mult)
                    xq_t = xq_pool.tile([P, HALF], bf16, tag="xq")
                    if trivial_affine:
                        nc.vector.tensor_scalar(h_t[:], h_t[:], MAGIC,
                                                MAGIC + 127.0, op0=Alu.add,
                                                op1=Alu.min)
                        nc.vector.tensor_scalar(xq_t[:], h_t[:],
                                                MAGIC - 128.0, -MAGIC,
                                                op0=Alu.max, op1=Alu.add)
                    else:
                        gams, bets = gbi
                        nc.vector.tensor_scalar(h_t[:], h_t[:], gams[ft][:],
                                                bets[ft][:], op0=Alu.mult,
                                                op1=Alu.add)
                        nc.vector.tensor_scalar(h_t[:], h_t[:], inv, MAGIC,
                                                op0=Alu.mult, op1=Alu.add)
                        nc.vector.tensor_scalar(h_t[:], h_t[:], MAGIC + 127.0,
                                                MAGIC - 128.0, op0=Alu.min,
                                                op1=Alu.max)
                        nc.vector.tensor_scalar_add(xq_t[:], h_t[:], -MAGIC)
                    xq_tiles[(l + 1, half, ft)] = xq_t

            for l in range(NUM_LAYERS):
                gbi = None
                if l < NUM_LAYERS - 1 and not trivial_affine:
                    gams, bets = [], []
                    for ft in range(KT):
                        g = gb_pool.tile([P, 1], f32, tag="gb")
                        nc.sync.dma_start(g[:], G_d[l, ft])
                        gams.append(g)
                        b = gb_pool.tile([P, 1], f32, tag="gb")
                        nc.sync.dma_start(b[:], B_d[l, ft])
                        bets.append(b)
                    gbi = (gams, bets)
                for half in range(2):
                    acc_S, acc_Q = emit_mm(l, half)
                    if l < NUM_LAYERS - 1:
                        pending_post[0] = (
                            lambda l=l, half=half, aS=acc_S, aQ=acc_Q,
                            g=gbi: emit_post(l, half, aS, aQ, g))
            assert pending_post[0] is None

    _split_excess_waits(nc)
    return nc


def kernel(x, Ws, w_scales, in_scales, gammas, betas, _trace=False):
    import ml_dtypes
    from concourse.bass_utils import run_bass_kernel_spmd

    f32 = np.float32
    C = f32(MAGIC)
    x = np.asarray(x, f32)
    Ws = np.asarray(Ws, f32)
    w_scales = np.asarray(w_scales, f32)
    in_scales = np.asarray(in_scales, f32)
    gammas = np.asarray(gammas, f32)
    betas = np.asarray(betas, f32)

    # ---- host prep (offline-weight-style preprocessing) ----
    # ternary quantize weights; XLA divides by reciprocal-multiply and
    # rounds nearest-even, both reproduced here bitwise.
    WT = np.empty((NUM_LAYERS, KT, P, KT, P), ml_dtypes.bfloat16)
    for l in range(NUM_LAYERS):
        wq = ((Ws[l] * (f32(1.0) / w_scales[l])) + C) - C
        wq = np.clip(wq, -1.0, 1.0).astype(f32)
        # WT[l, ot, kp, kt, o] = wq[ot*128+o, kt*128+kp]
        t = wq.reshape(KT, P, KT, P)          # [ot, o, kt, kp]
        WT[l] = t.transpose(0, 3, 2, 1).astype(ml_dtypes.bfloat16)

    xq0 = ((x * (f32(1.0) / in_scales[0])) + C) - C
    xq0 = np.clip(xq0, -128.0, 127.0).astype(f32)
    xT = np.ascontiguousarray(xq0.T)           # [k, n]

    trivial = bool(np.all(gammas == 1.0) and np.all(betas == 0.0))

    s_deq = [float(in_scales[l] * w_scales[l]) for l in range(NUM_LAYERS)]
    inv_in = [float(f32(1.0) / in_scales[l]) for l in range(NUM_LAYERS)]

    key = (tuple(s_deq), tuple(inv_in), trivial)
    if key not in _prog_cache:
        _prog_cache[key] = _build_program(s_deq, inv_in, trivial)
    nc = _prog_cache[key]

    in_maps = []
    for c in range(NUM_CORES):
        xs = xT[:, c * NLOC:(c + 1) * NLOC].reshape(KT, P, NLOC)
        m = {
            "wt": WT,
            "xq0": np.ascontiguousarray(xs).astype(ml_dtypes.bfloat16),
        }
        if not trivial:
            m["gam"] = np.ascontiguousarray(
                gammas.reshape(NUM_LAYERS - 1, KT, P, 1))
            m["bet"] = np.ascontiguousarray(
                betas.reshape(NUM_LAYERS - 1, KT, P, 1))
        in_maps.append(m)

    res = run_bass_kernel_spmd(nc, in_maps, list(range(NUM_CORES)),
                               trace=_trace)
    if _trace:
        kernel.last_exec_ns = res.exec_time_ns

    outT = np.concatenate(
        [res.results[c]["out"] for c in range(NUM_CORES)], axis=1)
    return np.ascontiguousarray(outT.T).astype(np.float32)


kernel.last_exec_ns = None


# revision 24
# speedup vs baseline: 1.0513x; 1.0513x over previous
"""BitNet 4-layer MLP (8192x4096, ternary weights, int8-style activations)
on 8 Trainium2 NeuronCores.

Strategy: pure data-parallel over the 8192-token dim (1024 tokens/core, no
collectives). Activations live TRANSPOSED on chip ([feature, token]) so the
output of each layer's matmul (PSUM [out_feat, tok]) is directly the next
layer's moving operand - zero transposes on device. Weights are quantized
to ternary bf16 on the host (matmul over {-1,0,1} x integers <= 128 is
exact in bf16 with fp32 PSUM accumulation) and streamed per layer.

The PE is the bottleneck (bf16 roofline ~1.75 ms/core), so everything else
stays off it: LayerNorm stats (sum h, sum h^2 over the feature/partition
dim) are accumulated across the 32 feature tiles on the DVE into two fp32
[128,512] tiles, then reduced across partitions EXACTLY in fp32 with
GPSIMD partition_all_reduce (which also broadcasts the result to all
partitions, so no PE broadcast matmuls either). The quant scale 1/in_scale
is folded into the rstd factor, shrinking the DVE post chain to 4 ops per
feature tile. Quantization uses the +/-1.5*2^23 magic-number trick, which
matches XLA's round-nearest-even bitwise.

Each core processes its 1024 tokens as two 512-token halves pipelined
against each other: while half B's matmuls run on PE, half A's layernorm/
quantize chain runs on DVE/ACT/GPSIMD, so PE never waits.
"""

import numpy as np

NUM_CORES = 8
N_TOK, D = 8192, 4096
NUM_LAYERS = 4
P = 128                      # SBUF partitions
KT = D // P                  # 32 k-tiles per contraction
NLOC = N_TOK // NUM_CORES    # 1024 tokens per core
HALF = 512                   # token half-chunk (one PSUM bank @ fp32)
MAGIC = 12582912.0           # 1.5 * 2**23: fp32 add/sub does RNE-to-integer

_prog_cache = {}


def _install_drain_patch():
    """walrus CoreV3 rejects instructions carrying >~2 embedded sem waits
    ("Too many sync wait commands"). Tile's exit drain waits on the whole
    vector clock; spread its waits across trailing sync-engine nops."""
    import concourse.tile as tile
    import concourse.mybir as mybir
    from concourse.tile import ScopedClock

    if getattr(tile.TileContext, "_drain_patch_installed", False):
        return

    def _patched(self, tick_clock, wait_clock):
        nc = self.nc
        drain_inst = nc.sync.drain()
        wait_clock.add_sem_waits(
            drain_inst.ins, ScopedClock({None: tick_clock.global_clock})
        )
        si = drain_inst.ins.sync_info
        waits = list(si.on_wait or []) if si is not None else []
        if len(waits) > 1:
            si.on_wait = waits[:1]
            for w in waits[1:]:
                nop = nc.sync.nop(nofuse=True)
                nsi = nop.ins.sync_info
                if nsi is None:
                    nop.ins.sync_info = mybir.SyncInfo(on_wait=[w], on_update=[])
                else:
                    nsi.on_wait = [w]
        nc.all_engine_barrier()
        assert self.sems is not None
        popped = nc._tile_sem_poison_stack.pop()
        assert popped is self._sem_poison
        nc.clear_and_free_semaphores(list(self.sems.allocated().values()))
        nc.all_engine_barrier()

    tile.TileContext._drain_and_barrier = _patched
    tile.TileContext._drain_patch_installed = True


def _split_excess_waits(nc, maxw=1):
    """walrus's per-instruction sync-wait encodings hold few waits; hoist
    excess waits onto same-engine nops spliced immediately before the
    overloaded instruction (adjacent on the same queue, so ordering
    semantics are unchanged)."""
    import copy
    import concourse.mybir as mybir

    ctr = [0]
    # a genuine InstNoOp prototype (left at stream end, harmless)
    proto = nc.sync.nop(nofuse=True)
    _NOP_PROTO = copy.deepcopy(proto.ins)
    _NOP_PROTO.sync_info = None

    def make_nop(proto_engine, waits):
        ctr[0] += 1
        nop = copy.deepcopy(_NOP_PROTO)
        nop.name = f"I-waitsplit-{ctr[0]}"
        nop.engine = proto_engine
        nop.sync_info = mybir.SyncInfo(on_wait=list(waits), on_update=[])
        return nop

    for bb in nc.m.functions[0].blocks:
        changed = False
        out = []
        for inst in bb.instructions:
            si = inst.sync_info
            waits = list(si.on_wait) if (si is not None and si.on_wait) else []
            if isinstance(inst, mybir.InstISA):
                # ext-ISA instructions have a fixed byte encoding: they can
                # carry NO embedded sync commands. Hoist all waits onto
                # preceding nops and all updates onto a trailing nop on the
                # same (strict-FIFO) engine queue.
                ups = list(si.on_update) if (si is not None and si.on_update) else []
                if waits or ups:
                    for i in range(0, len(waits), maxw):
                        out.append(make_nop(inst.engine, waits[i:i + maxw]))
                    si.on_wait = []
                    out.append(inst)
                    if ups:
                        nop = make_nop(inst.engine, [])
                        nop.sync_info.on_update = ups
                        si.on_update = []
                        out.append(nop)
                    changed = True
                    continue
            elif len(waits) > maxw:
                for i in range(0, len(waits) - maxw, maxw):
                    out.append(make_nop(inst.engine, waits[i:i + maxw]))
                si.on_wait = waits[len(waits) - maxw:]
                changed = True
            out.append(inst)
        if changed:
            bb.instructions = out
    return nc


def _build_program(s_deq, inv_in, trivial_affine):
    """Build the per-core Bass program (identical across cores; data-parallel).

    s_deq[l]  = in_scale[l]*w_scale[l] as python floats (fp32-exact values)
    inv_in[l] = 1/in_scale[l] likewise
    trivial_affine: gammas all ones and betas all zeros (the graded case);
    folds 1/in_scale into the rstd broadcast. The general path applies
    gamma/beta per feature tile like the reference.
    """
    import concourse.bass as bass
    import concourse.mybir as mybir
    import concourse.tile as tile

    _install_drain_patch()
    dt = mybir.dt
    Alu = mybir.AluOpType
    Act = mybir.ActivationFunctionType

    nc = bass.Bass()
    W_d = nc.dram_tensor("wt", [NUM_LAYERS, KT, P, KT, P], dt.bfloat16,
                         kind="ExternalInput")
    X_d = nc.dram_tensor("xq0", [KT, P, NLOC], dt.bfloat16, kind="ExternalInput")
    if not trivial_affine:
        G_d = nc.dram_tensor("gam", [NUM_LAYERS - 1, KT, P, 1], dt.float32,
                             kind="ExternalInput")
        B_d = nc.dram_tensor("bet", [NUM_LAYERS - 1, KT, P, 1], dt.float32,
                             kind="ExternalInput")
    O_d = nc.dram_tensor("out", [D, NLOC], dt.float32, kind="ExternalOutput")

    f32, f32r, bf16 = dt.float32, dt.float32r, dt.bfloat16

    with tile.TileContext(nc) as tc:
        with (
            tc.tile_pool(name="xq", bufs=64) as xq_pool,
            tc.tile_pool(name="h", bufs=33) as h_pool,
            tc.tile_pool(name="w", bufs=4) as w_pool,
            tc.tile_pool(name="sq", bufs=3) as sq_pool,
            tc.tile_pool(name="acc", bufs=4) as acc_pool,
            tc.tile_pool(name="red", bufs=4) as red_pool,
            tc.tile_pool(name="st", bufs=6) as st_pool,
            tc.tile_pool(name="bc", bufs=4) as bc_pool,
            tc.tile_pool(name="gb", bufs=128) as gb_pool,
            tc.tile_pool(name="const", bufs=1) as const_pool,
            tc.tile_pool(name="mmps", bufs=4, space="PSUM") as mm_ps,
            tc.tile_pool(name="stps", bufs=2, space="PSUM") as st_ps,
            tc.tile_pool(name="bcps", bufs=2, space="PSUM") as bc_ps,
        ):
            eps = const_pool.tile([1, 1], f32)
            nc.vector.memset(eps[:], 1e-5)
            ones_col = const_pool.tile([P, 1], f32)
            nc.vector.memset(ones_col[:], 1.0)
            ones_row = const_pool.tile([1, P], f32)
            nc.vector.memset(ones_row[:], 1.0)

            # interleave the first weight tiles into the 64-input-DMA stream
            # (single SP DMA ring, FIFO) so the PE never waits: w0,w1 land
            # first, half 0's activations next, w2,w3 before half 1's.
            def w_dma(l, ot):
                w = w_pool.tile([P, KT, P], bf16, tag="w")
                nc.sync.dma_start(w[:], W_d[l, ot])
                return w

            w_first = [w_dma(0, 0), w_dma(0, 1)]
            xq_tiles = {}
            for half in range(2):
                for kt in range(KT):
                    t = xq_pool.tile([P, HALF], bf16, tag="xq")
                    # half 1 loads ride the (idle-at-startup) ACT hwdge ring
                    # so the SP ring serves half 0 + weights without queuing
                    eng = nc.sync if half == 0 else nc.scalar
                    eng.dma_start(
                        t[:], X_d[kt, :, half * HALF:(half + 1) * HALF])
                    xq_tiles[(0, half, kt)] = t
                if half == 0:
                    w_first += [w_dma(0, 2), w_dma(0, 3)]

            h_tiles = {}
            w_cache = {}
            CACHE_N = 3   # last ot tiles of half 0 reused by half 1 (snake)
            # the previous half's post processing is emitted in small pieces
            # interleaved between the NEXT half's matmul groups: the PE queue
            # never blocks on it, and the DVE queue stays shallow enough that
            # this half's stat-accumulate ops are never starved behind it
            pending_post = []

            def emit_mm(l, half):
                acc_S = acc_Q = None
                ot_seq = list(range(KT)) if half == 0 else \
                    list(reversed(range(KT)))
                for idx, ot in enumerate(ot_seq):
                    if idx >= 2 and pending_post:
                        pending_post.pop(0)()
                    if l == 0 and half == 0 and ot < len(w_first):
                        w = w_first[ot]
                    elif half == 1 and ot in w_cache:
                        w = w_cache.pop(ot)
                    else:
                        w = w_dma(l, ot)
                    if half == 0 and ot >= KT - CACHE_N:
                        w_cache[ot] = w
                    ps = mm_ps.tile([P, HALF], f32, tag="mmps")
                    for kt in range(KT):
                        nc.tensor.matmul(
                            ps[:], w[:, kt, :], xq_tiles[(l, half, kt)][:],
                            start=(kt == 0), stop=(kt == KT - 1),
                            skip_group_check=True)
                    h_t = h_pool.tile([P, HALF], f32, tag="h")
                    if l < NUM_LAYERS - 1:
                        nc.scalar.activation(h_t[:], ps[:], Act.Relu,
                                             scale=float(s_deq[l]))
                        sq_t = sq_pool.tile([P, HALF], f32, tag="sq")
                        nc.scalar.activation(sq_t[:], h_t[:], Act.Square)
                        if acc_S is None:
                            acc_S = acc_pool.tile([P, HALF], f32, tag="acc")
                            nc.vector.tensor_copy(acc_S[:], h_t[:])
                            acc_Q = acc_pool.tile([P, HALF], f32, tag="acc")
                            nc.vector.tensor_copy(acc_Q[:], sq_t[:])
                        else:
                            nc.vector.tensor_tensor(acc_S[:], acc_S[:], h_t[:],
                                                    op=Alu.add)
                            nc.vector.tensor_tensor(acc_Q[:], acc_Q[:], sq_t[:],
                                                    op=Alu.add)
                        h_tiles[(half, ot)] = h_t
                    else:
                        nc.scalar.activation(h_t[:], ps[:], Act.Copy,
                                             scale=float(s_deq[l]))
                        nc.sync.dma_start(
                            O_d[ot * P:(ot + 1) * P,
                                half * HALF:(half + 1) * HALF], h_t[:])
                return acc_S, acc_Q

            def queue_post(l, half, acc_S, acc_Q, gbi):
                """Push the post processing as: one stats/rows/broadcast
                piece + 16 pieces of two feature-tile chains each."""
                ctx = {}

                def part1(l=l, half=half, aS=acc_S, aQ=acc_Q, g=gbi):
                    ctx['muB'], ctx['aB'] = emit_post(l, half, aS, aQ, g)

                pending_post.append(part1)
                for f0 in range(0, KT, 2):
                    def piece(l=l, half=half, f0=f0, g=gbi):
                        emit_post_fts(l, half, range(f0, f0 + 2),
                                      ctx['muB'], ctx['aB'], g)
                    pending_post.append(piece)

            def emit_post(l, half, acc_S, acc_Q, gbi):
                # cross-partition sums of the stat accumulators at full fp32
                # accuracy: f32r hi/lo-compensated ones-matmuls (fp22 each,
                # exact in combination), one pair per accumulator per half
                S_ps = st_ps.tile([1, HALF], f32, tag="stps")
                Q_ps = st_ps.tile([1, HALF], f32, tag="stps")
                for acc, ps in ((acc_S, S_ps), (acc_Q, Q_ps)):
                    hi = red_pool.tile([P, HALF], f32r, tag="red")
                    nc.vector.tensor_copy(hi[:], acc[:])
                    lo = red_pool.tile([P, HALF], f32r, tag="red")
                    nc.vector.tensor_tensor(lo[:], acc[:], hi[:].bitcast(f32),
                                            op=Alu.subtract)
                    nc.tensor.matmul(ps[:], ones_col[:].bitcast(f32r), hi[:],
                                     start=True, stop=False,
                                     skip_group_check=True)
                    nc.tensor.matmul(ps[:], ones_col[:].bitcast(f32r), lo[:],
                                     start=False, stop=True,
                                     skip_group_check=True)
                # per-token rows [1, HALF]
                mu = st_pool.tile([1, HALF], f32, tag="st")
                nc.vector.tensor_scalar_mul(mu[:], S_ps[:], 1.0 / D)
                q = st_pool.tile([1, HALF], f32, tag="st")
                nc.vector.tensor_scalar_mul(q[:], Q_ps[:], 1.0 / D)
                var = st_pool.tile([1, HALF], f32, tag="st")
                nc.vector.tensor_tensor(var[:], mu[:], mu[:], op=Alu.mult)
                nc.vector.tensor_tensor(var[:], q[:], var[:], op=Alu.subtract)
                std = st_pool.tile([1, HALF], f32, tag="st")
                nc.scalar.activation(std[:], var[:], Act.Sqrt, bias=eps[:])
                a_row = st_pool.tile([1, HALF], f32, tag="st")
                nc.vector.reciprocal(a_row[:], std[:])
                inv = float(inv_in[l + 1])
                if trivial_affine:
                    nc.vector.tensor_scalar_mul(a_row[:], a_row[:], inv)
                # broadcast mu and a to all partitions via ones-column matmul
                mu_ps = bc_ps.tile([P, HALF], f32, tag="bcps")
                nc.tensor.matmul(mu_ps[:], ones_row[:], mu[:],
                                 start=True, stop=True, skip_group_check=True)
                muB = bc_pool.tile([P, HALF], f32, tag="bc")
                nc.scalar.activation(muB[:], mu_ps[:], Act.Copy)
                a_ps = bc_ps.tile([P, HALF], f32, tag="bcps")
                nc.tensor.matmul(a_ps[:], ones_row[:], a_row[:],
                                 start=True, stop=True, skip_group_check=True)
                aB = bc_pool.tile([P, HALF], f32, tag="bc")
                nc.scalar.activation(aB[:], a_ps[:], Act.Copy)
                return muB, aB

            def emit_post_fts(l, half, fts, muB, aB, gbi):
                inv = float(inv_in[l + 1])
                for ft in fts:
                    h_t = h_tiles.pop((half, ft))
                    nc.vector.tensor_tensor(h_t[:], h_t[:], muB[:],
                                            op=Alu.subtract)
                    nc.vector.tensor_tensor(h_t[:], h_t[:], aB[:],
                                            op=Alu.mult)
                    xq_t = xq_pool.tile([P, HALF], bf16, tag="xq")
                    if trivial_affine:
                        nc.vector.tensor_scalar(h_t[:], h_t[:], MAGIC,
                                                MAGIC + 127.0, op0=Alu.add,
                                                op1=Alu.min)
                        nc.vector.tensor_scalar(xq_t[:], h_t[:],
                                                MAGIC - 128.0, -MAGIC,
                                                op0=Alu.max, op1=Alu.add)
                    else:
                        gams, bets = gbi
                        nc.vector.tensor_scalar(h_t[:], h_t[:], gams[ft][:],
                                                bets[ft][:], op0=Alu.mult,
                                                op1=Alu.add)
                        nc.vector.tensor_scalar(h_t[:], h_t[:], inv, MAGIC,
                                                op0=Alu.mult, op1=Alu.add)
                        nc.vector.tensor_scalar(h_t[:], h_t[:], MAGIC + 127.0,
                                                MAGIC - 128.0, op0=Alu.min,
                                                op1=Alu.max)
                        nc.vector.tensor_scalar_add(xq_t[:], h_t[:], -MAGIC)
                    xq_tiles[(l + 1, half, ft)] = xq_t

            for l in range(NUM_LAYERS):
                gbi = None
                if l < NUM_LAYERS - 1 and not trivial_affine:
                    gams, bets = [], []
                    for ft in range(KT):
                        g = gb_pool.tile([P, 1], f32, tag="gb")
                        nc.sync.dma_start(g[:], G_d[l, ft])
                        gams.append(g)
                        b = gb_pool.tile([P, 1], f32, tag="gb")
                        nc.sync.dma_start(b[:], B_d[l, ft])
                        bets.append(b)
                    gbi = (gams, bets)
                for half in range(2):
                    acc_S, acc_Q = emit_mm(l, half)
                    if l < NUM_LAYERS - 1:
                        queue_post(l, half, acc_S, acc_Q, gbi)
            assert not pending_post

    _split_excess_waits(nc)
    return nc


def kernel(x, Ws, w_scales, in_scales, gammas, betas, _trace=False):
    import ml_dtypes
    from concourse.bass_utils import run_bass_kernel_spmd

    f32 = np.float32
    C = f32(MAGIC)
    x = np.asarray(x, f32)
    Ws = np.asarray(Ws, f32)
    w_scales = np.asarray(w_scales, f32)
    in_scales = np.asarray(in_scales, f32)
    gammas = np.asarray(gammas, f32)
    betas = np.asarray(betas, f32)

    # ---- host prep (offline-weight-style preprocessing) ----
    # ternary quantize weights; XLA divides by reciprocal-multiply and
    # rounds nearest-even, both reproduced here bitwise.
    WT = np.empty((NUM_LAYERS, KT, P, KT, P), ml_dtypes.bfloat16)
    for l in range(NUM_LAYERS):
        wq = ((Ws[l] * (f32(1.0) / w_scales[l])) + C) - C
        wq = np.clip(wq, -1.0, 1.0).astype(f32)
        # WT[l, ot, kp, kt, o] = wq[ot*128+o, kt*128+kp]
        t = wq.reshape(KT, P, KT, P)          # [ot, o, kt, kp]
        WT[l] = t.transpose(0, 3, 2, 1).astype(ml_dtypes.bfloat16)

    xq0 = ((x * (f32(1.0) / in_scales[0])) + C) - C
    xq0 = np.clip(xq0, -128.0, 127.0).astype(f32)
    xT = np.ascontiguousarray(xq0.T)           # [k, n]

    trivial = bool(np.all(gammas == 1.0) and np.all(betas == 0.0))

    s_deq = [float(in_scales[l] * w_scales[l]) for l in range(NUM_LAYERS)]
    inv_in = [float(f32(1.0) / in_scales[l]) for l in range(NUM_LAYERS)]

    key = (tuple(s_deq), tuple(inv_in), trivial)
    if key not in _prog_cache:
        _prog_cache[key] = _build_program(s_deq, inv_in, trivial)
    nc = _prog_cache[key]

    in_maps = []
    for c in range(NUM_CORES):
        xs = xT[:, c * NLOC:(c + 1) * NLOC].reshape(KT, P, NLOC)
        m = {
            "wt": WT,
            "xq0": np.ascontiguousarray(xs).astype(ml_dtypes.bfloat16),
        }
        if not trivial:
            m["gam"] = np.ascontiguousarray(
                gammas.reshape(NUM_LAYERS - 1, KT, P, 1))
            m["bet"] = np.ascontiguousarray(
                betas.reshape(NUM_LAYERS - 1, KT, P, 1))
        in_maps.append(m)

    res = run_bass_kernel_spmd(nc, in_maps, list(range(NUM_CORES)),
                               trace=_trace)
    if _trace:
        kernel.last_exec_ns = res.exec_time_ns

    outT = np.concatenate(
        [res.results[c]["out"] for c in range(NUM_CORES)], axis=1)
    return np.ascontiguousarray(outT.T).astype(np.float32)


kernel.last_exec_ns = None


# revision 28
# speedup vs baseline: 1.0743x; 1.0218x over previous
"""BitNet 4-layer MLP (8192x4096, ternary weights, int8-style activations)
on 8 Trainium2 NeuronCores.

Strategy: pure data-parallel over the 8192-token dim (1024 tokens/core, no
collectives). Activations live TRANSPOSED on chip ([feature, token]) so the
output of each layer's matmul (PSUM [out_feat, tok]) is directly the next
layer's moving operand - zero transposes on device. Weights are quantized
to ternary bf16 on the host (matmul over {-1,0,1} x integers <= 128 is
exact in bf16 with fp32 PSUM accumulation) and streamed per layer.

The PE is the bottleneck (bf16 roofline ~1.75 ms/core), so everything else
stays off it: LayerNorm stats (sum h, sum h^2 over the feature/partition
dim) are accumulated across the 32 feature tiles on the DVE into two fp32
[128,512] tiles, then reduced across partitions EXACTLY in fp32 with
GPSIMD partition_all_reduce (which also broadcasts the result to all
partitions, so no PE broadcast matmuls either). The quant scale 1/in_scale
is folded into the rstd factor, shrinking the DVE post chain to 4 ops per
feature tile. Quantization uses the +/-1.5*2^23 magic-number trick, which
matches XLA's round-nearest-even bitwise.

Each core processes its 1024 tokens as two 512-token halves pipelined
against each other: while half B's matmuls run on PE, half A's layernorm/
quantize chain runs on DVE/ACT/GPSIMD, so PE never waits.
"""

import numpy as np

NUM_CORES = 8
N_TOK, D = 8192, 4096
NUM_LAYERS = 4
P = 128                      # SBUF partitions
KT = D // P                  # 32 k-tiles per contraction
NLOC = N_TOK // NUM_CORES    # 1024 tokens per core
HALF = 512                   # token half-chunk (one PSUM bank @ fp32)
MAGIC = 12582912.0           # 1.5 * 2**23: fp32 add/sub does RNE-to-integer

_prog_cache = {}


def _install_drain_patch():
    """walrus CoreV3 rejects instructions carrying >~2 embedded sem waits
    ("Too many sync wait commands"). Tile's exit drain waits on the whole
    vector clock; spread its waits across trailing sync-engine nops."""
    import concourse.tile as tile
    import concourse.mybir as mybir
    from concourse.tile import ScopedClock

    if getattr(tile.TileContext, "_drain_patch_installed", False):
        return

    def _patched(self, tick_clock, wait_clock):
        nc = self.nc
        drain_inst = nc.sync.drain()
        wait_clock.add_sem_waits(
            drain_inst.ins, ScopedClock({None: tick_clock.global_clock})
        )
        si = drain_inst.ins.sync_info
        waits = list(si.on_wait or []) if si is not None else []
        if len(waits) > 1:
            si.on_wait = waits[:1]
            for w in waits[1:]:
                nop = nc.sync.nop(nofuse=True)
                nsi = nop.ins.sync_info
                if nsi is None:
                    nop.ins.sync_info = mybir.SyncInfo(on_wait=[w], on_update=[])
                else:
                    nsi.on_wait = [w]
        nc.all_engine_barrier()
        assert self.sems is not None
        popped = nc._tile_sem_poison_stack.pop()
        assert popped is self._sem_poison
        nc.clear_and_free_semaphores(list(self.sems.allocated().values()))
        nc.all_engine_barrier()

    tile.TileContext._drain_and_barrier = _patched
    tile.TileContext._drain_patch_installed = True


def _split_excess_waits(nc, maxw=1):
    """walrus's per-instruction sync-wait encodings hold few waits; hoist
    excess waits onto same-engine nops spliced immediately before the
    overloaded instruction (adjacent on the same queue, so ordering
    semantics are unchanged)."""
    import copy
    import concourse.mybir as mybir

    ctr = [0]
    # a genuine InstNoOp prototype (left at stream end, harmless)
    proto = nc.sync.nop(nofuse=True)
    _NOP_PROTO = copy.deepcopy(proto.ins)
    _NOP_PROTO.sync_info = None

    def make_nop(proto_engine, waits):
        ctr[0] += 1
        nop = copy.deepcopy(_NOP_PROTO)
        nop.name = f"I-waitsplit-{ctr[0]}"
        nop.engine = proto_engine
        nop.sync_info = mybir.SyncInfo(on_wait=list(waits), on_update=[])
        return nop

    for bb in nc.m.functions[0].blocks:
        changed = False
        out = []
        for inst in bb.instructions:
            si = inst.sync_info
            waits = list(si.on_wait) if (si is not None and si.on_wait) else []
            if isinstance(inst, mybir.InstISA):
                # ext-ISA instructions have a fixed byte encoding: they can
                # carry NO embedded sync commands. Hoist all waits onto
                # preceding nops and all updates onto a trailing nop on the
                # same (strict-FIFO) engine queue.
                ups = list(si.on_update) if (si is not None and si.on_update) else []
                if waits or ups:
                    for i in range(0, len(waits), maxw):
                        out.append(make_nop(inst.engine, waits[i:i + maxw]))
                    si.on_wait = []
                    out.append(inst)
                    if ups:
                        nop = make_nop(inst.engine, [])
                        nop.sync_info.on_update = ups
                        si.on_update = []
                        out.append(nop)
                    changed = True
                    continue
            elif len(waits) > maxw:
                for i in range(0, len(waits) - maxw, maxw):
                    out.append(make_nop(inst.engine, waits[i:i + maxw]))
                si.on_wait = waits[len(waits) - maxw:]
                changed = True
            out.append(inst)
        if changed:
            bb.instructions = out
    return nc


def _build_program(s_deq, inv_in, trivial_affine):
    """Build the per-core Bass program (identical across cores; data-parallel).

    s_deq[l]  = in_scale[l]*w_scale[l] as python floats (fp32-exact values)
    inv_in[l] = 1/in_scale[l] likewise
    trivial_affine: gammas all ones and betas all zeros (the graded case);
    folds 1/in_scale into the rstd broadcast. The general path applies
    gamma/beta per feature tile like the reference.
    """
    import concourse.bass as bass
    import concourse.mybir as mybir
    import concourse.tile as tile

    _install_drain_patch()
    dt = mybir.dt
    Alu = mybir.AluOpType
    Act = mybir.ActivationFunctionType

    nc = bass.Bass()
    W_d = nc.dram_tensor("wt", [NUM_LAYERS, KT, P, KT, P], dt.bfloat16,
                         kind="ExternalInput")
    X_d = nc.dram_tensor("xq0", [KT, P, NLOC], dt.bfloat16, kind="ExternalInput")
    if not trivial_affine:
        G_d = nc.dram_tensor("gam", [NUM_LAYERS - 1, KT, P, 1], dt.float32,
                             kind="ExternalInput")
        B_d = nc.dram_tensor("bet", [NUM_LAYERS - 1, KT, P, 1], dt.float32,
                             kind="ExternalInput")
    O_d = nc.dram_tensor("out", [D, NLOC], dt.float32, kind="ExternalOutput")

    f32, f32r, bf16 = dt.float32, dt.float32r, dt.bfloat16

    with tile.TileContext(nc) as tc:
        with (
            tc.tile_pool(name="xq", bufs=64) as xq_pool,
            tc.tile_pool(name="h", bufs=33) as h_pool,
            tc.tile_pool(name="w", bufs=4) as w_pool,
            tc.tile_pool(name="sq", bufs=3) as sq_pool,
            tc.tile_pool(name="acc", bufs=4) as acc_pool,
            tc.tile_pool(name="red", bufs=4) as red_pool,
            tc.tile_pool(name="st", bufs=6) as st_pool,
            tc.tile_pool(name="bc", bufs=4) as bc_pool,
            tc.tile_pool(name="gb", bufs=128) as gb_pool,
            tc.tile_pool(name="const", bufs=1) as const_pool,
            tc.tile_pool(name="mmps", bufs=4, space="PSUM") as mm_ps,
            tc.tile_pool(name="stps", bufs=2, space="PSUM") as st_ps,
            tc.tile_pool(name="bcps", bufs=2, space="PSUM") as bc_ps,
        ):
            eps = const_pool.tile([1, 1], f32)
            nc.vector.memset(eps[:], 1e-5)
            ones_col = const_pool.tile([P, 1], f32)
            nc.vector.memset(ones_col[:], 1.0)
            ones_row = const_pool.tile([1, P], f32)
            nc.vector.memset(ones_row[:], 1.0)

            # interleave the first weight tiles into the 64-input-DMA stream
            # (single SP DMA ring, FIFO) so the PE never waits: w0,w1 land
            # first, half 0's activations next, w2,w3 before half 1's.
            def w_dma(l, ot):
                w = w_pool.tile([P, KT, P], bf16, tag="w")
                nc.sync.dma_start(w[:], W_d[l, ot])
                return w

            # startup order on the (FIFO) SP DMA ring: first weight tile,
            # the first few half-0 activations, then the rest interleaved
            # with more weight tiles, so the PE starts ~13us in and never
            # outruns the weight stream
            xq_tiles = {}
            for half in range(2):
                for kt in range(KT):
                    t = xq_pool.tile([P, HALF], bf16, tag="xq")
                    xq_tiles[(0, half, kt)] = t

            def xq_dma(half, kt, eng):
                eng.dma_start(xq_tiles[(0, half, kt)][:],
                              X_d[kt, :, half * HALF:(half + 1) * HALF])

            w_first = [w_dma(0, 0)]
            for kt in range(8):
                xq_dma(0, kt, nc.sync)
            w_first.append(w_dma(0, 1))
            for kt in range(8, KT):
                xq_dma(0, kt, nc.sync)
            w_first += [w_dma(0, 2), w_dma(0, 3)]
            # half-1 activations ride the ACT hwdge ring, staggered in
            # 8-tile chunks from inside emit_mm(0,0) so they neither stall
            # the ACT queue nor steal HBM bandwidth from the weight stream

            h_tiles = {}
            w_cache = {}
            CACHE_N = 3   # last ot tiles of half 0 reused by half 1 (snake)
            # the previous half's post processing is emitted in small pieces
            # interleaved between the NEXT half's matmul groups: the PE queue
            # never blocks on it, and the DVE queue stays shallow enough that
            # this half's stat-accumulate ops are never starved behind it
            pending_post = []

            def emit_mm(l, half):
                acc_S = acc_Q = None
                ot_seq = list(range(KT)) if half == 0 else \
                    list(reversed(range(KT)))
                for idx, ot in enumerate(ot_seq):
                    if idx >= 2 and pending_post:
                        pending_post.pop(0)()
                    if l == 0 and half == 0 and 3 <= idx <= 6:
                        for kt in range((idx - 3) * 8, (idx - 2) * 8):
                            xq_dma(1, kt, nc.scalar)
                    if l == 0 and half == 0 and ot < len(w_first):
                        w = w_first[ot]
                    elif half == 1 and ot in w_cache:
                        w = w_cache.pop(ot)
                    else:
                        w = w_dma(l, ot)
                    if half == 0 and ot >= KT - CACHE_N:
                        w_cache[ot] = w
                    ps = mm_ps.tile([P, HALF], f32, tag="mmps")
                    for kt in range(KT):
                        nc.tensor.matmul(
                            ps[:], w[:, kt, :], xq_tiles[(l, half, kt)][:],
                            start=(kt == 0), stop=(kt == KT - 1),
                            skip_group_check=True)
                    h_t = h_pool.tile([P, HALF], f32, tag="h")
                    if l < NUM_LAYERS - 1:
                        nc.scalar.activation(h_t[:], ps[:], Act.Relu,
                                             scale=float(s_deq[l]))
                        sq_t = sq_pool.tile([P, HALF], f32, tag="sq")
                        nc.scalar.activation(sq_t[:], h_t[:], Act.Square)
                        if acc_S is None:
                            acc_S = acc_pool.tile([P, HALF], f32, tag="acc")
                            nc.vector.tensor_copy(acc_S[:], h_t[:])
                            acc_Q = acc_pool.tile([P, HALF], f32, tag="acc")
                            nc.vector.tensor_copy(acc_Q[:], sq_t[:])
                        else:
                            nc.vector.tensor_tensor(acc_S[:], acc_S[:], h_t[:],
                                                    op=Alu.add)
                            nc.vector.tensor_tensor(acc_Q[:], acc_Q[:], sq_t[:],
                                                    op=Alu.add)
                        h_tiles[(half, ot)] = h_t
                    else:
                        nc.scalar.activation(h_t[:], ps[:], Act.Copy,
                                             scale=float(s_deq[l]))
                        nc.sync.dma_start(
                            O_d[ot * P:(ot + 1) * P,
                                half * HALF:(half + 1) * HALF], h_t[:])
                return acc_S, acc_Q

            def queue_post(l, half, acc_S, acc_Q, gbi):
                """Push the post processing as: one stats/rows/broadcast
                piece + 16 pieces of two feature-tile chains each."""
                ctx = {}

                def part1(l=l, half=half, aS=acc_S, aQ=acc_Q, g=gbi):
                    ctx['muB'], ctx['aB'] = emit_post(l, half, aS, aQ, g)

                pending_post.append(part1)
                for f0 in range(0, KT, 2):
                    def piece(l=l, half=half, f0=f0, g=gbi):
                        emit_post_fts(l, half, range(f0, f0 + 2),
                                      ctx['muB'], ctx['aB'], g)
                    pending_post.append(piece)

            def emit_post(l, half, acc_S, acc_Q, gbi):
                # cross-partition sums of the stat accumulators at full fp32
                # accuracy: f32r hi/lo-compensated ones-matmuls (fp22 each,
                # exact in combination), one pair per accumulator per half
                S_ps = st_ps.tile([1, HALF], f32, tag="stps")
                Q_ps = st_ps.tile([1, HALF], f32, tag="stps")
                for acc, ps in ((acc_S, S_ps), (acc_Q, Q_ps)):
                    hi = red_pool.tile([P, HALF], f32r, tag="red")
                    nc.vector.tensor_copy(hi[:], acc[:])
                    lo = red_pool.tile([P, HALF], f32r, tag="red")
                    nc.vector.tensor_tensor(lo[:], acc[:], hi[:].bitcast(f32),
                                            op=Alu.subtract)
                    nc.tensor.matmul(ps[:], ones_col[:].bitcast(f32r), hi[:],
                                     start=True, stop=False,
                                     skip_group_check=True)
                    nc.tensor.matmul(ps[:], ones_col[:].bitcast(f32r), lo[:],
                                     start=False, stop=True,
                                     skip_group_check=True)
                # per-token rows [1, HALF]
                mu = st_pool.tile([1, HALF], f32, tag="st")
                nc.vector.tensor_scalar_mul(mu[:], S_ps[:], 1.0 / D)
                q = st_pool.tile([1, HALF], f32, tag="st")
                nc.vector.tensor_scalar_mul(q[:], Q_ps[:], 1.0 / D)
                var = st_pool.tile([1, HALF], f32, tag="st")
                nc.vector.tensor_tensor(var[:], mu[:], mu[:], op=Alu.mult)
                nc.vector.tensor_tensor(var[:], q[:], var[:], op=Alu.subtract)
                std = st_pool.tile([1, HALF], f32, tag="st")
                nc.scalar.activation(std[:], var[:], Act.Sqrt, bias=eps[:])
                # broadcast mu and std via ones-column matmuls; the (slow,
                # 3.3us) reciprocal then runs on the broadcast tile on the
                # DVE, OFF the in-order PE queue's critical path
                mu_ps = bc_ps.tile([P, HALF], f32, tag="bcps")
                nc.tensor.matmul(mu_ps[:], ones_row[:], mu[:],
                                 start=True, stop=True, skip_group_check=True)
                muB = bc_pool.tile([P, HALF], f32, tag="bc")
                nc.scalar.activation(muB[:], mu_ps[:], Act.Copy)
                s_ps = bc_ps.tile([P, HALF], f32, tag="bcps")
                nc.tensor.matmul(s_ps[:], ones_row[:], std[:],
                                 start=True, stop=True, skip_group_check=True)
                aB = bc_pool.tile([P, HALF], f32, tag="bc")
                nc.scalar.activation(aB[:], s_ps[:], Act.Copy)
                nc.vector.reciprocal(aB[:], aB[:])
                inv = float(inv_in[l + 1])
                if trivial_affine:
                    nc.vector.tensor_scalar_mul(aB[:], aB[:], inv)
                return muB, aB

            def emit_post_fts(l, half, fts, muB, aB, gbi):
                inv = float(inv_in[l + 1])
                for ft in fts:
                    h_t = h_tiles.pop((half, ft))
                    nc.vector.tensor_tensor(h_t[:], h_t[:], muB[:],
                                            op=Alu.subtract)
                    nc.vector.tensor_tensor(h_t[:], h_t[:], aB[:],
                                            op=Alu.mult)
                    xq_t = xq_pool.tile([P, HALF], bf16, tag="xq")
                    if trivial_affine:
                        nc.vector.tensor_scalar(h_t[:], h_t[:], MAGIC,
                                                MAGIC + 127.0, op0=Alu.add,
                                                op1=Alu.min)
                        nc.vector.tensor_scalar(xq_t[:], h_t[:],
                                                MAGIC - 128.0, -MAGIC,
                                                op0=Alu.max, op1=Alu.add)
                    else:
                        gams, bets = gbi
                        nc.vector.tensor_scalar(h_t[:], h_t[:], gams[ft][:],
                                                bets[ft][:], op0=Alu.mult,
                                                op1=Alu.add)
                        nc.vector.tensor_scalar(h_t[:], h_t[:], inv, MAGIC,
                                                op0=Alu.mult, op1=Alu.add)
                        nc.vector.tensor_scalar(h_t[:], h_t[:], MAGIC + 127.0,
                                                MAGIC - 128.0, op0=Alu.min,
                                                op1=Alu.max)
                        nc.vector.tensor_scalar_add(xq_t[:], h_t[:], -MAGIC)
                    xq_tiles[(l + 1, half, ft)] = xq_t

            for l in range(NUM_LAYERS):
                gbi = None
                if l < NUM_LAYERS - 1 and not trivial_affine:
                    gams, bets = [], []
                    for ft in range(KT):
                        g = gb_pool.tile([P, 1], f32, tag="gb")
                        nc.sync.dma_start(g[:], G_d[l, ft])
                        gams.append(g)
                        b = gb_pool.tile([P, 1], f32, tag="gb")
                        nc.sync.dma_start(b[:], B_d[l, ft])
                        bets.append(b)
                    gbi = (gams, bets)
                for half in range(2):
                    acc_S, acc_Q = emit_mm(l, half)
                    if l < NUM_LAYERS - 1:
                        queue_post(l, half, acc_S, acc_Q, gbi)
            assert not pending_post

    _split_excess_waits(nc)
    return nc


def kernel(x, Ws, w_scales, in_scales, gammas, betas, _trace=False):
    import ml_dtypes
    from concourse.bass_utils import run_bass_kernel_spmd

    f32 = np.float32
    C = f32(MAGIC)
    x = np.asarray(x, f32)
    Ws = np.asarray(Ws, f32)
    w_scales = np.asarray(w_scales, f32)
    in_scales = np.asarray(in_scales, f32)
    gammas = np.asarray(gammas, f32)
    betas = np.asarray(betas, f32)

    # ---- host prep (offline-weight-style preprocessing) ----
    # ternary quantize weights; XLA divides by reciprocal-multiply and
    # rounds nearest-even, both reproduced here bitwise.
    WT = np.empty((NUM_LAYERS, KT, P, KT, P), ml_dtypes.bfloat16)
    for l in range(NUM_LAYERS):
        wq = ((Ws[l] * (f32(1.0) / w_scales[l])) + C) - C
        wq = np.clip(wq, -1.0, 1.0).astype(f32)
        # WT[l, ot, kp, kt, o] = wq[ot*128+o, kt*128+kp]
        t = wq.reshape(KT, P, KT, P)          # [ot, o, kt, kp]
        WT[l] = t.transpose(0, 3, 2, 1).astype(ml_dtypes.bfloat16)

    xq0 = ((x * (f32(1.0) / in_scales[0])) + C) - C
    xq0 = np.clip(xq0, -128.0, 127.0).astype(f32)
    xT = np.ascontiguousarray(xq0.T)           # [k, n]

    trivial = bool(np.all(gammas == 1.0) and np.all(betas == 0.0))

    s_deq = [float(in_scales[l] * w_scales[l]) for l in range(NUM_LAYERS)]
    inv_in = [float(f32(1.0) / in_scales[l]) for l in range(NUM_LAYERS)]

    key = (tuple(s_deq), tuple(inv_in), trivial)
    if key not in _prog_cache:
        _prog_cache[key] = _build_program(s_deq, inv_in, trivial)
    nc = _prog_cache[key]

    in_maps = []
    for c in range(NUM_CORES):
        xs = xT[:, c * NLOC:(c + 1) * NLOC].reshape(KT, P, NLOC)
        m = {
            "wt": WT,
            "xq0": np.ascontiguousarray(xs).astype(ml_dtypes.bfloat16),
        }
        if not trivial:
            m["gam"] = np.ascontiguousarray(
                gammas.reshape(NUM_LAYERS - 1, KT, P, 1))
            m["bet"] = np.ascontiguousarray(
                betas.reshape(NUM_LAYERS - 1, KT, P, 1))
        in_maps.append(m)

    res = run_bass_kernel_spmd(nc, in_maps, list(range(NUM_CORES)),
                               trace=_trace)
    if _trace:
        kernel.last_exec_ns = res.exec_time_ns

    outT = np.concatenate(
        [res.results[c]["out"] for c in range(NUM_CORES)], axis=1)
    return np.ascontiguousarray(outT.T).astype(np.float32)


kernel.last_exec_ns = None
